# revision 13
# baseline (speedup 1.0000x reference)
"""Trainium2 Bass kernel for nn_DecoderLSTM (30-step decoder LSTM, npeds=8192,
hidden=256, embed=64), data-parallel over peds across 8 NeuronCores.

Restructured v2 (vs. 841us baseline):
  - PE: per gate-chunk output [128,512]: W_hh contribution as either two bf16
    K=128 matmuls or ONE fp8 DoubleRow matmul (K=256, same cost as one bf16
    MM), plus one K=65 matmul that folds the LSTM bias via a constant ones
    row in the dec-input tile.  Gate biases are NOT applied on ACT.
  - ACT: one merged sigmoid over {i,f,o} [128,1536] per chunk + tanh(g) +
    tanh(c) per chunk; tail sigmoids merged to one [32,32] op.  PRELU of the
    embedding moved to DVE (scalar_tensor_tensor max).  Single table set.
  - Cross-step software pipelining: PE FIFO order is
      ... gatesA(t) gatesB(t) [statsA(t) embA(t)] gatesA(t+1)
          [statsB(t) embB(t)] gatesB(t+1) ...
    so the tensor engine never drains while the A/B tail chains run ->
    HAM stays at K=8/8 (the baseline oscillated between 1.2/2.4 GHz).
  - Stats: 4 accumulating matmuls (A-chunks on h, 1/H rows on h^2); the
    eps and -mu^2 terms are folded into the tail's scalar_tensor_tensor ops.
  - Output rel values are written as [32,32] blocks (no back-transpose);
    host reassembles.
  - PSUM: 2x [128,3,512] buffers for {i,f,o} + 2x [128,512] rotating
    buffers shared by {g, stats, embed} = exactly 8 banks.
"""
import os
import sys

for _p in ("/root/.axon_site/_ro/trn_rl_repo", "/opt/trn_rl_repo"):
    if os.path.isdir(_p) and _p not in sys.path:
        sys.path.insert(0, _p)

import numpy as np
import ml_dtypes

import concourse.bass as bass
import concourse.tile as tile
from concourse import bacc, mybir
from concourse import bass_utils
from concourse.bass_interp import get_hw_module


def _ensure_ntff_hook_module():
    """Provide antenv.axon_hooks if the image ships without it, so
    run_bass_kernel_spmd(trace=True) can capture NTFF profiles."""
    try:
        from antenv import axon_hooks  # noqa: F401
        return
    except ImportError:
        pass
    import types

    mod = types.ModuleType("antenv.axon_hooks")
    mod._HOOK = None

    def set_axon_ntff_profile_hook(hook):
        mod._HOOK = hook

    def get_axon_ntff_profile_hook():
        if mod._HOOK is None:
            try:
                from trn_agent_boot.trn_boot import _ntff_profile_via_ctypes
                mod._HOOK = _ntff_profile_via_ctypes("/opt/axon/libaxon_pjrt.so")
            except Exception:
                mod._HOOK = None
        return mod._HOOK

    mod.set_axon_ntff_profile_hook = set_axon_ntff_profile_hook
    mod.get_axon_ntff_profile_hook = get_axon_ntff_profile_hook
    sys.modules["antenv.axon_hooks"] = mod
    try:
        import antenv
        antenv.axon_hooks = mod
    except ImportError:
        pass


_ensure_ntff_hook_module()

F32 = mybir.dt.float32
BF16 = mybir.dt.bfloat16
FP8 = mybir.dt.float8e4
I32 = mybir.dt.int32
DT = BF16              # elementwise / weight dtype
AF = mybir.ActivationFunctionType
OP = mybir.AluOpType
PM = mybir.MatmulPerfMode

USE_FP8 = False        # fp8 DoubleRow for the W_hh matmuls (h state in fp8)
DT_H = FP8 if USE_FP8 else BF16

N_CORES = 8
NPEDS = 8192
NP_CORE = NPEDS // N_CORES      # 1024
HALF = NP_CORE // 2             # 512
H = 256
E = 64
T = 30
EPS = 1e-5
LEAK = 0.01
MAGIC = 0x5F3759DF
TANH_S_SCALE = 0.88 / float(np.sqrt(4.0 * EPS))


def _build_program():
    nc = bacc.Bacc(
        "TRN2",
        target_bir_lowering=False,
        debug=False,
        enable_asserts=False,
        num_devices=N_CORES,
    )

    # ---- DRAM I/O ----
    d = {}
    if USE_FP8:
        d["LWQ"] = nc.dram_tensor("LWQ", [128, 8, 2, 128], FP8, kind="ExternalInput")
    else:
        d["LW1"] = nc.dram_tensor("LW1", [128, 1024], DT, kind="ExternalInput")
        d["LW2"] = nc.dram_tensor("LW2", [128, 1024], DT, kind="ExternalInput")
    d["LWD"] = nc.dram_tensor("LWD", [128, 1024], DT, kind="ExternalInput")
    d["STW"] = nc.dram_tensor("STW", [128, 128], DT_H, kind="ExternalInput")
    d["WE2"] = nc.dram_tensor("WE2", [2, 64], DT, kind="ExternalInput")
    d["PB0"] = nc.dram_tensor("PB0", [32, 1], F32, kind="ExternalInput")
    d["PB1"] = nc.dram_tensor("PB1", [32, 1], F32, kind="ExternalInput")
    d["DEC0"] = nc.dram_tensor("DEC0", [128, 1024], DT, kind="ExternalInput")
    d["H0"] = nc.dram_tensor("H0", [128, 2048], DT_H, kind="ExternalInput")
    d["C0"] = nc.dram_tensor("C0", [128, 2048], DT, kind="ExternalInput")
    out_t = nc.dram_tensor("OUT", [T, 2, 32, 32], F32, kind="ExternalOutput")

    with tile.TileContext(nc) as tc:
        with (
            tc.tile_pool(name="weights", bufs=1) as wp,
            tc.tile_pool(name="state", bufs=1) as sp,
            tc.tile_pool(name="acts", bufs=2) as ap_,
            tc.tile_pool(name="dve", bufs=2) as dp,
            tc.tile_pool(name="tailp", bufs=2) as tp,
            tc.tile_pool(name="pifo", bufs=2, space="PSUM") as pifo,
            tc.tile_pool(name="paux", bufs=1, space="PSUM") as paux,
        ):
            # ---- persistent weights in SBUF ----
            if USE_FP8:
                LWQ = wp.tile([128, 8, 2, 128], FP8, tag="LWQ")
                nc.sync.dma_start(LWQ[:], d["LWQ"].ap())
            else:
                LW1 = wp.tile([128, 1024], DT, tag="LW1")
                LW2 = wp.tile([128, 1024], DT, tag="LW2")
                nc.sync.dma_start(LW1[:], d["LW1"].ap())
                nc.sync.dma_start(LW2[:], d["LW2"].ap())
            LWD = wp.tile([128, 1024], DT, tag="LWD")
            STW = wp.tile([128, 128], DT_H, tag="STW")
            WE2 = wp.tile([2, 64], DT, tag="WE2")
            PB0 = wp.tile([32, 1], F32, tag="PB0")
            PB1 = wp.tile([32, 1], F32, tag="PB1")
            IONE = wp.tile([32, 16], I32, tag="IONE")
            IMAGIC = wp.tile([32, 16], I32, tag="IMAGIC")
            for name, tl in [("LWD", LWD), ("STW", STW), ("WE2", WE2),
                             ("PB0", PB0), ("PB1", PB1)]:
                nc.sync.dma_start(tl[:], d[name].ap())
            nc.vector.memset(IONE[:], 1)
            nc.vector.memset(IMAGIC[:], MAGIC)

            # ---- persistent state: [half][parity] ----
            HQ = [[sp.tile([128, 2, HALF], DT_H, name=f"HQ_{h}_{p}",
                           tag=f"HQ_{h}_{p}") for p in range(2)]
                  for h in range(2)]
            CB = [[sp.tile([128, 2, HALF], DT, name=f"CB_{h}_{p}",
                           tag=f"CB_{h}_{p}") for p in range(2)]
                  for h in range(2)]
            # dec-input stack: row0 = s, row1 = 1, rows 2:64 = 0, 64:128 = |z|
            T0 = [[sp.tile([128, HALF], DT, name=f"T0_{h}_{p}",
                           tag=f"T0_{h}_{p}") for p in range(2)]
                  for h in range(2)]
            HSQ = [sp.tile([128, 2, HALF], DT_H, name=f"HSQ_{h}",
                           tag=f"HSQ_{h}") for h in range(2)]
            # s staging: [:, :, 0] = s values, [:, :, 1] = 1, rest 0
            SSB = [sp.tile([32, 16, 32], DT, name=f"SSB_{h}",
                           tag=f"SSB_{h}") for h in range(2)]
            for h in range(2):
                nc.vector.memset(SSB[h][:], 0.0)
                nc.vector.memset(SSB[h][:, :, 1], 1.0)
                nc.vector.memset(T0[h][1][32:64, :], 0.0)

            for h in range(2):
                nc.sync.dma_start(T0[h][0][:], d["DEC0"].ap()[:, h * HALF:(h + 1) * HALF])
                nc.sync.dma_start(HQ[h][0][:], d["H0"].ap()[:, h * 1024:(h + 1) * 1024])
                nc.sync.dma_start(CB[h][0][:], d["C0"].ap()[:, h * 1024:(h + 1) * 1024])

            GATE_BLK = {"i": 0, "f": 1, "o": 2, "g": 3}

            def h_mms(out, blk, h, p, start=True):
                if USE_FP8:
                    nc.tensor.matmul(out, LWQ[:, blk], HQ[h][p][:],
                                     start=start, stop=False,
                                     perf_mode=PM.DoubleRow)
                else:
                    nc.tensor.matmul(out, LW1[:, blk * 128:(blk + 1) * 128],
                                     HQ[h][p][:, 0], start=start, stop=False)
                    nc.tensor.matmul(out, LW2[:, blk * 128:(blk + 1) * 128],
                                     HQ[h][p][:, 1], start=False, stop=False)

            def dec_mm(out, blk, h, p):
                nc.tensor.matmul(out, LWD[:, blk * 128:(blk + 1) * 128],
                                 T0[h][p][:], start=False, stop=True)

            def stats_mms(h, t_prev, STEB):
                """4 accumulating matmuls: [num0,num1,mu,eh2] rows of STEB[0:32]."""
                pp = (t_prev + 1) % 2
                st32 = STEB[0:32, 0]
                nc.tensor.matmul(st32, STW[:, 0:32], HQ[h][pp][:, 0],
                                 start=True, stop=False)
                nc.tensor.matmul(st32, STW[:, 32:64], HQ[h][pp][:, 1],
                                 start=False, stop=False)
                nc.tensor.matmul(st32, STW[:, 64:96], HSQ[h][:, 0],
                                 start=False, stop=False)
                nc.tensor.matmul(st32, STW[:, 96:128], HSQ[h][:, 1],
                                 start=False, stop=True)

            def tail_chain(h, t_prev, STEB, final=False):
                """LN2 + pos head for (h, t_prev): rel out + next dec stack."""
                pp = (t_prev + 1) % 2
                tailT = tp.tile([32, 16, 32], F32, name="tailT", tag="tailT")
                nc.vector.transpose(tailT[:], STEB[0:32, 0])
                num0 = tailT[:, :, 0]
                num1 = tailT[:, :, 1]
                mu = tailT[:, :, 2]
                eh2 = tailT[:, :, 3]

                mu2 = dp.tile([32, 16], F32, name="mu2", tag="mu2")
                nc.vector.scalar_tensor_tensor(mu2[:], mu, -1.0, mu,
                                               OP.mult, OP.mult)
                V = dp.tile([32, 16], F32, name="V", tag="V")
                nc.vector.scalar_tensor_tensor(V[:], eh2, EPS, mu2[:],
                                               OP.add, OP.add)
                # rsqrt via int bit-trick + 1 Newton step
                y = dp.tile([32, 16], F32, name="y", tag="y")
                sh = dp.tile([32, 16], I32, name="sh", tag="sh")
                nc.vector.tensor_tensor(sh[:], V.bitcast(I32)[:], IONE[:],
                                        OP.arith_shift_right)
                nc.vector.tensor_tensor(y.bitcast(I32)[:], IMAGIC[:], sh[:],
                                        OP.subtract)
                a = dp.tile([32, 16], F32, name="a", tag="a")
                nc.vector.tensor_tensor(a[:], y[:], y[:], OP.mult)
                nc.vector.scalar_tensor_tensor(a[:], a[:], -0.5, V[:],
                                               OP.mult, OP.mult)
                r = dp.tile([32, 16], F32, name="r", tag="r")
                nc.vector.scalar_tensor_tensor(r[:], a[:], 1.5, y[:],
                                               OP.add, OP.mult)

                z = tp.tile([32, 32], F32, name="z", tag="z")
                nc.vector.tensor_tensor(z[:, 0:16], num0, r[:], OP.mult)
                nc.vector.tensor_tensor(z[:, 16:32], num1, r[:], OP.mult)
                nc.vector.tensor_scalar(z[:, 0:16], z[:, 0:16], PB0[:], None,
                                        OP.add)
                nc.vector.tensor_scalar(z[:, 16:32], z[:, 16:32], PB1[:], None,
                                        OP.add)
                relS = tp.tile([32, 32], F32, name="relS", tag="relS")
                nc.scalar.activation(relS[:], z[:], AF.Sigmoid)
                nc.sync.dma_start(out_t.ap()[t_prev][h], relS[:])
                if final:
                    return
                e = dp.tile([32, 16], F32, name="e", tag="e")
                nc.vector.tensor_tensor(e[:], relS[:, 0:16], relS[:, 16:32],
                                        OP.subtract)
                nc.scalar.activation(SSB[h][:, :, 0], e[:], AF.Tanh,
                                     scale=TANH_S_SCALE)
                # s/ones rows straight into the dec stack (junk rows x 0 wts)
                nc.vector.transpose(T0[h][pp][0:32, :], SSB[h][:])
                # embed matmul z = w*s + b into psum partitions 64:128
                nc.tensor.matmul(STEB[64:128, 0], WE2[0:2, :],
                                 T0[h][pp][0:2, :], start=True, stop=True,
                                 tile_position=(0, 64))
                nc.scalar.activation(T0[h][pp][64:128, :],
                                     STEB[64:128, 0], AF.Abs)

            def half_step(h, t):
                p, q = t % 2, (t + 1) % 2
                P0 = pifo.tile([128, 3, HALF], F32, name="P0", tag="ifo")
                h_mms(P0[:, 0, :], 8 * 0 + GATE_BLK["i"], h, p)
                h_mms(P0[:, 1, :], GATE_BLK["f"], h, p)
                STEB = paux.tile([128, 2, HALF], F32, name="STEB", tag="aux")
                if t > 0:
                    stats_mms(h, t - 1, STEB)
                h_mms(P0[:, 2, :], GATE_BLK["o"], h, p)
                P1 = pifo.tile([128, 3, HALF], F32, name="P1", tag="ifo")
                h_mms(P1[:, 0, :], 4 + GATE_BLK["i"], h, p)
                h_mms(P1[:, 1, :], 4 + GATE_BLK["f"], h, p)
                h_mms(P1[:, 2, :], 4 + GATE_BLK["o"], h, p)
                if t > 0:
                    tail_chain(h, t - 1, STEB)
                G = paux.tile([128, 2, HALF], F32, name="G", tag="aux")
                h_mms(G[:, 0, :], GATE_BLK["g"], h, p)
                h_mms(G[:, 1, :], 4 + GATE_BLK["g"], h, p)
                # dec matmuls close the accumulation groups
                dec_mm(P0[:, 0, :], GATE_BLK["i"], h, p)
                dec_mm(P0[:, 1, :], GATE_BLK["f"], h, p)
                dec_mm(P0[:, 2, :], GATE_BLK["o"], h, p)
                dec_mm(G[:, 0, :], GATE_BLK["g"], h, p)
                dec_mm(G[:, 1, :], 4 + GATE_BLK["g"], h, p)
                dec_mm(P1[:, 0, :], 4 + GATE_BLK["i"], h, p)
                dec_mm(P1[:, 1, :], 4 + GATE_BLK["f"], h, p)
                dec_mm(P1[:, 2, :], 4 + GATE_BLK["o"], h, p)
                # activations
                SIG = [None, None]
                for ch, P in enumerate((P0, P1)):
                    SIG[ch] = ap_.tile([128, 3, HALF], DT, name="SIG", tag="sig")
                    nc.scalar.activation(SIG[ch][:], P[:], AF.Sigmoid)
                GT = ap_.tile([128, 2, HALF], DT, name="GT", tag="gt")
                nc.scalar.activation(GT[:], G[:], AF.Tanh)
                # cell update
                for ch in range(2):
                    m2 = dp.tile([128, HALF], DT, name="m2", tag="m2")
                    nc.vector.tensor_tensor(m2[:], SIG[ch][:, 0], GT[:, ch],
                                            OP.mult)
                    m1 = dp.tile([128, HALF], DT, name="m1", tag="m1")
                    nc.vector.tensor_tensor(m1[:], SIG[ch][:, 1],
                                            CB[h][p][:, ch], OP.mult)
                    nc.vector.tensor_tensor(CB[h][q][:, ch], m1[:], m2[:],
                                            OP.add)
                TC = ap_.tile([128, 2, HALF], DT, name="TC", tag="tc")
                nc.scalar.activation(TC[:], CB[h][q][:], AF.Tanh)
                for ch in range(2):
                    hn = HQ[h][q][:, ch]
                    nc.vector.tensor_tensor(hn, SIG[ch][:, 2], TC[:, ch],
                                            OP.mult)
                    nc.vector.tensor_tensor(HSQ[h][:, ch], hn, hn, OP.mult)

            # ---- software-pipelined time loop ----
            for t in range(T):
                half_step(0, t)
                half_step(1, t)
            for h in range(2):
                STEB = paux.tile([128, 2, HALF], F32, name="STEB", tag="aux")
                stats_mms(h, T - 1, STEB)
                tail_chain(h, T - 1, STEB, final=True)

    nc.compile()
    return nc


_NC_CACHE = None


def _get_program():
    global _NC_CACHE
    if _NC_CACHE is None:
        _NC_CACHE = _build_program()
    return _NC_CACHE


def _prepare_in_maps(inputs):
    f32 = np.float32
    inp = {k: np.asarray(v, f32) for k, v in inputs.items()}
    W_ih, W_hh = inp["W_ih"], inp["W_hh"]
    bias = (inp["b_ih"] + inp["b_hh"]).astype(f32)

    # gate-row permutation: per chunk [i, f, o, g] (torch layout i,f,g,o)
    perm = []
    for chunk in range(2):
        for base in (0, H, 3 * H, 2 * H):          # i, f, o, g
            start = base + chunk * 128
            perm.extend(range(start, start + 128))
    perm = np.array(perm)
    Wih_p, Whh_p, bias_p = W_ih[perm], W_hh[perm], bias[perm]

    LW1 = np.ascontiguousarray(Whh_p[:, 0:128].T)          # [128, 1024]
    LW2 = np.ascontiguousarray(Whh_p[:, 128:256].T)        # [128, 1024]

    emb_W, emb_b = inp["emb_W"], inp["emb_b"]
    g1, b1 = inp["ln1_g"], inp["ln1_b"]
    w_emb = (g1[0] * emb_W[:, 0] - g1[1] * emb_W[:, 1]).astype(f32)
    emb_bp = (emb_b + b1[0] * emb_W[:, 0] + b1[1] * emb_W[:, 1]).astype(f32)

    # dec-input stack weights: prelu(z) = 0.505 z + 0.495 |z|, z = w*s + b;
    # W_ih @ dec = 0.505 (W_ih w) s + 0.505 (W_ih b) 1 + 0.495 W_ih |z|
    ALO = (1.0 + LEAK) / 2.0
    AHI = (1.0 - LEAK) / 2.0
    LWD = np.zeros((128, 1024), f32)
    LWD[0] = ALO * (Wih_p @ w_emb)
    LWD[1] = bias_p + ALO * (Wih_p @ emb_bp)
    LWD[64:128] = AHI * Wih_p.T

    pos_W, pos_b = inp["pos_W"], inp["pos_b"]
    g2, b2 = inp["ln2_g"], inp["ln2_b"]
    posWp = (pos_W * g2[None, :]).astype(f32)
    pos_bp = (pos_b + b2 @ pos_W.T).astype(f32)
    w1 = posWp.sum(1)
    A = posWp - w1[:, None] / H                            # [2, 256]

    STW = np.zeros((128, 128), f32)
    STW[:, 0], STW[:, 1], STW[:, 2] = A[0, 0:128], A[1, 0:128], 1.0 / H
    STW[:, 32], STW[:, 33], STW[:, 34] = A[0, 128:256], A[1, 128:256], 1.0 / H
    STW[:, 64 + 3] = 1.0 / H
    STW[:, 96 + 3] = 1.0 / H

    WE2 = np.stack([w_emb, emb_bp], 0)                     # [2, 64]

    lpr = inp["last_pos_rel"]
    e0 = lpr[:, 0] - lpr[:, 1]
    s0 = e0 / np.sqrt(e0 * e0 + 4 * EPS)
    z0 = s0[:, None] * w_emb[None, :] + emb_bp[None, :]    # [N, 64]
    dec0T = np.zeros((128, NPEDS), f32)
    dec0T[0] = s0
    dec0T[1] = 1.0
    dec0T[64:128] = np.abs(z0).T

    h0T = np.ascontiguousarray(inp["h0"][0].T)             # [256, N]
    c0T = np.ascontiguousarray(inp["c0"][0].T)

    bf = ml_dtypes.bfloat16
    f8 = ml_dtypes.float8_e4m3
    hdt = f8 if USE_FP8 else bf
    rep = {
        "LWD": LWD.astype(bf),
        "STW": STW.astype(hdt), "WE2": WE2.astype(bf),
        "PB0": np.full((32, 1), pos_bp[0], f32),
        "PB1": np.full((32, 1), pos_bp[1], f32),
    }
    if USE_FP8:
        LWQ = np.zeros((128, 8, 2, 128), f32)
        for blk in range(8):
            LWQ[:, blk, 0, :] = LW1[:, blk * 128:(blk + 1) * 128]
            LWQ[:, blk, 1, :] = LW2[:, blk * 128:(blk + 1) * 128]
        rep["LWQ"] = LWQ.astype(f8)
    else:
        rep["LW1"] = LW1.astype(bf)
        rep["LW2"] = LW2.astype(bf)

    in_maps = []
    for c in range(N_CORES):
        cols = slice(c * NP_CORE, (c + 1) * NP_CORE)
        dec_c = dec0T[:, cols]
        h_c, c_c = h0T[:, cols], c0T[:, cols]
        # [128, 2048]: per half: [chunk0 512 | chunk1 512]
        H0 = np.zeros((128, 2048), f32)
        C0 = np.zeros((128, 2048), f32)
        for h in range(2):
            for ch in range(2):
                dst = slice(h * 1024 + ch * 512, h * 1024 + (ch + 1) * 512)
                H0[:, dst] = h_c[ch * 128:(ch + 1) * 128, h * 512:(h + 1) * 512]
                C0[:, dst] = c_c[ch * 128:(ch + 1) * 128, h * 512:(h + 1) * 512]
        m = dict(rep)
        m["DEC0"] = np.ascontiguousarray(dec_c).astype(bf)
        m["H0"] = H0.astype(hdt)
        m["C0"] = C0.astype(bf)
        in_maps.append(m)
    return in_maps


def _assemble(raw):
    """raw: [T, 2, 32, 32] per core -> [T, NP_CORE, 2]."""
    r = raw.reshape(T, 2, 32, 2, 16)          # t, half, row, j, block
    r = r.transpose(0, 1, 4, 2, 3)            # t, half, block, row, j
    return np.ascontiguousarray(r.reshape(T, NP_CORE, 2))


def run_on_hw(inputs, trace=False, **kwargs):
    nc = _get_program()
    in_maps = _prepare_in_maps(inputs)
    old_m = nc.m
    nc.m = get_hw_module(nc.m)
    try:
        res = bass_utils.run_bass_kernel_spmd(
            nc, in_maps, core_ids=list(range(N_CORES)), trace=trace, **kwargs)
    finally:
        nc.m = old_m
    out = np.concatenate([_assemble(np.asarray(r["OUT"], np.float32))
                          for r in res.results], axis=1)
    return out.astype(np.float32), res


def kernel(**inputs) -> np.ndarray:
    out, _ = run_on_hw(inputs, trace=False)
    return out
